# revision 44
# baseline (speedup 1.0000x reference)
"""Trainium2 Bass kernel for Mistral-style sliding-window GQA attention.

Problem: B=2, T=2048, C=2048, 32 q heads / 8 kv heads, head_dim=64,
sliding causal window 1024, RoPE, fp32.

Sharding (sequence-parallel, no cross-core communication):
  core c in 0..7 handles batch b=c//4 and contiguous 512-row chunk k=c%4.
  Each core computes q for its 512 rows, k/v for its rows plus a 1024-row
  halo (zero-padded before t=0), full attention for its rows over all 32
  heads, and the output projection for its rows.  Host gathers by
  concatenation only.

Device program (v4):
  - All four projections run as fp8e4m3 DoubleRow matmuls with a 3-term
    hi+lo error split (x_hi@W_hi + x_lo@W_hi + x_hi@W_lo; the dropped
    lo@lo term is ~0.1% relative).  Operands are pre-scaled (x*8, W*512)
    into e4m3's normal range; the 2^-12 product scale is folded into the
    rope stage copies, the v validity column (4096), and the output
    flush.  Each DoubleRow matmul contracts 256 rows at half the
    per-column cost, so projections cost 3/8 of their fp16 equivalent.
  - PV is reoriented: PT ([keys, q]) is the stationary operand (full
    128x128 array) and v_ext [keys, 65] moves, so each kv block costs 65
    columns instead of 512.  The accumulator lands in [q, d] layout with
    the softmax denominator in column 64; normalization is a
    per-partition reciprocal + tensor_scalar (no partition_broadcast),
    then a x8-identity matmul transposes back to [d, q] and the result
    is split on-chip into fp8 hi/lo for the o-projection.
  - Head dims are PAIR-INTERLEAVED on host for the RoPE rotate-half
    (adjacent-partition stream_shuffle), as in v3.  QK keeps fp16
    operands (fp8 scores fail the softmax error budget).
  - One fixed 8-bank PSUM choreography via six tags: B2a/B2b are 2-bank
    [128,1024] tiles (KV v psums, attention STs, o-proj rounds); B1a-d
    are 1-bank tiles (KV k psums, PV accumulators, transpose bank,
    q-proj bank, o-proj chains).
  - Q-projection DoubleRow steps are dispensed one per attention lk-step
    to plug the exp bubbles (ACT is the binding engine in the attention
    region); the oc0 o-proj chains dribble into the final attention pair
    the same way.  Score matmuls are emitted one step ahead of PV so a
    PV stalled on the previous unit's normalize never delays the next
    exp; per-unit transposes + fp8 splits are deferred into the next
    unit's early lk slots.
"""

import numpy as np
import ml_dtypes

import concourse.bass as bass
import concourse.mybir as mybir
import concourse.tile as tile
from concourse import bacc
from concourse.bass_utils import run_bass_kernel_spmd

B, T, C = 2, 2048, 2048
NH, NKV, D = 32, 8, 64
REP = NH // NKV
WIN = 1024
CH = 512          # q rows per core
KVR = CH + WIN    # kv rows per core (with halo)
NCORE = 8
DT = mybir.dt.float32
F16 = mybir.dt.float16
BF16 = mybir.dt.bfloat16
F8 = mybir.dt.float8e4
SCALE = 1.0 / np.sqrt(np.float32(D))
ROPE_BASE = 10000.0

NP2 = C // 256    # 8 contraction pair-chunks
NQT = CH // 128   # 4 q tiles per chunk
NKB = KVR // 128  # 12 kv blocks per core
NWB = 9           # kv blocks in the window of one q tile
VW = 65           # v_ext width per kv block (64 dims + validity column)
VP = NKB * VW     # per-head v_ext pitch (780)

SX = 8.0          # x pre-scale into e4m3 range
SW = 512.0        # weight pre-scale
SPS = 1.0 / (SX * SW)   # 2^-12 psum descale

# Head dims are PAIR-INTERLEAVED on host (new dim 2j = old j, 2j+1 = old
# j+32), so RoPE rotate-half is an adjacent-partition swap - expressible as
# a stream_shuffle (which permutes within 32-partition quadrants only).
_PAIR_SHUF = [i ^ 1 for i in range(32)]

DR = mybir.MatmulPerfMode.DoubleRow


def _rope_write(nc, pool, out_ap, ps, cosw, ssinw, n, stage_dve=False):
    """out = ps*cos + rot_half(ps)*sin on a [128, n] 2-head-packed tile.

    ps is a PSUM f32 tile (carrying the 2^12 fp8 product scale; the
    tables carry 2^-12); one copy stages it to fp16 SBUF (ACT by
    default, DVE when the ACT queue is exp-loaded), then all multiplies/
    adds run on DVE in fp16.  ssinw rows carry the rotate-half signs
    (rows 0-31/64-95 negated) and any folded scale; cosw carries the same
    scale.  out_ap receives the fp16 result.
    """
    rs = pool.tile([128, n], F16, tag="rope_rs", name="rope_rs")
    # the stage copy also folds away the 2^12 fp8 product scale (folding
    # it into the fp16 tables instead would push them into denormals)
    if stage_dve:
        nc.vector.tensor_scalar(
            out=rs[:], in0=ps[:], scalar1=SPS, scalar2=None,
            op0=mybir.AluOpType.mult)
    else:
        nc.scalar.activation(rs[:], ps[:],
                             mybir.ActivationFunctionType.Copy, scale=SPS)
    # rotate-half = adjacent-partition swap in the pair-interleaved layout
    sw = pool.tile([128, n], F16, tag="rope_sw", name="rope_sw")
    nc.vector.stream_shuffle(sw[:], rs[:], _PAIR_SHUF)
    t2 = pool.tile([128, n], F16, tag="rope_t2", name="rope_t2")
    nc.vector.tensor_mul(t2[:], sw[:], ssinw[:])
    t1 = pool.tile([128, n], F16, tag="rope_t1", name="rope_t1")
    nc.vector.tensor_mul(t1[:], rs[:], cosw[:])
    nc.vector.tensor_add(out_ap, t1[:], t2[:])


def build_program():
    nc = bacc.Bacc("TRN2", target_bir_lowering=False, debug=False,
                   num_devices=NCORE)

    xh_d = nc.dram_tensor("xkv_hi", [128, NP2 * 2, KVR], F8,
                          kind="ExternalInput")
    xl_d = nc.dram_tensor("xkv_lo", [128, NP2 * 2, KVR], F8,
                          kind="ExternalInput")
    wqh_d = nc.dram_tensor("wq_hi", [128, NP2 * 2, NH * D], F8,
                           kind="ExternalInput")
    wql_d = nc.dram_tensor("wq_lo", [128, NP2 * 2, NH * D], F8,
                           kind="ExternalInput")
    wkh_d = nc.dram_tensor("wk_hi", [128, NP2 * 2, NKV * D], F8,
                           kind="ExternalInput")
    wkl_d = nc.dram_tensor("wk_lo", [128, NP2 * 2, NKV * D], F8,
                           kind="ExternalInput")
    wvh_d = nc.dram_tensor("wv_hi", [128, NP2 * 2, NKV * D], F8,
                           kind="ExternalInput")
    wvl_d = nc.dram_tensor("wv_lo", [128, NP2 * 2, NKV * D], F8,
                           kind="ExternalInput")
    woh_d = nc.dram_tensor("wo_hi", [128, NP2 * 2, C], F8,
                           kind="ExternalInput")
    wol_d = nc.dram_tensor("wo_lo", [128, NP2 * 2, C], F8,
                           kind="ExternalInput")
    rqc_d = nc.dram_tensor("rope_q_cos", [128, CH], F16, kind="ExternalInput")
    rqs_d = nc.dram_tensor("rope_q_sin", [128, CH], F16, kind="ExternalInput")
    rkc_d = nc.dram_tensor("rope_k_cos", [128, KVR], F16, kind="ExternalInput")
    rks_d = nc.dram_tensor("rope_k_sin", [128, KVR], F16, kind="ExternalInput")
    kvv_d = nc.dram_tensor("kvvalid", [128, NKB], BF16, kind="ExternalInput")
    mw_d = nc.dram_tensor("mask_win8", [128, 1024], BF16, kind="ExternalInput")
    mc_d = nc.dram_tensor("mask_causal8", [128, 1024], BF16,
                          kind="ExternalInput")
    id8_d = nc.dram_tensor("ident8", [128, 128], F16, kind="ExternalInput")
    mwu_d = nc.dram_tensor("mask_win_u8", [128, 512], mybir.dt.uint8,
                           kind="ExternalInput")
    out_d = nc.dram_tensor("out", [CH, C], F16, kind="ExternalOutput")

    with tile.TileContext(nc) as tc:
        with (
            tc.tile_pool(name="const", bufs=1) as cpool,
            tc.tile_pool(name="qT", bufs=1) as qT_pool,
            tc.tile_pool(name="kT", bufs=1) as kT_pool,
            tc.tile_pool(name="vext", bufs=1) as v_pool,
            tc.tile_pool(name="x2", bufs=1) as x2_pool,
            tc.tile_pool(name="ahi", bufs=1) as a_pool,
            tc.tile_pool(name="wq_s", bufs=1) as wq_pool,
            tc.tile_pool(name="xkv_s", bufs=12) as xkv_pool,
            tc.tile_pool(name="w8", bufs=32) as w_pool,
            tc.tile_pool(name="rope_tmp", bufs=2) as rtmp,
            tc.tile_pool(name="pt", bufs=4) as pt_pool,
            tc.tile_pool(name="att_small", bufs=3) as sm_pool,
            tc.tile_pool(name="ostage", bufs=2) as ostage,
            tc.tile_pool(name="psum", bufs=1, space="PSUM") as psum,
        ):
            # PSUM is hand-choreographed on 8 banks via six fixed tags (no
            # pool transitions, so no pipeline barriers anywhere).
            def b2(tag):
                return psum.tile([128, 1024], DT, tag=tag, name=tag)

            def b1(tag):
                return psum.tile([128, 512], DT, tag=tag, name=tag)

            B1 = ["B1a", "B1b", "B1c", "B1d"]
            st_ctr = [0]

            def next_b2():
                t = ["B2a", "B2b"][st_ctr[0] % 2]
                st_ctr[0] += 1
                return b2(t)

            def pv_acc(h):
                # [q, (r, 128)] accumulator inside one B1 bank: 128-word
                # pitch so each rep's accumulation group owns its own
                # 512-byte PSUM zero region (65-word pitch would put two
                # open groups in one zero region, which start=True zeroes)
                t = b1(B1[h])
                return t[:].rearrange("p (r w) -> p r w", r=REP)

            # ---- constants (small, persistent; SWDGE queue) ----
            mask_win = cpool.tile([128, 1024], BF16, tag="mw", name="mask_win")
            nc.gpsimd.dma_start(mask_win[:], mw_d[:, :])
            mask_causal = cpool.tile([128, 1024], BF16, tag="mc",
                                     name="mask_causal")
            nc.gpsimd.dma_start(mask_causal[:], mc_d[:, :])
            kvv = cpool.tile([128, NKB], BF16, tag="kvv", name="kvv")
            nc.gpsimd.dma_start(kvv[:], kvv_d[:, :])
            id8 = cpool.tile([128, 128], F16, tag="id8", name="id8")
            nc.gpsimd.dma_start(id8[:], id8_d[:, :])
            mwu = cpool.tile([128, 512], mybir.dt.uint8,
                             tag="mwu", name="mwu")
            nc.gpsimd.dma_start(mwu[:], mwu_d[:, :])
            rkc = cpool.tile([128, KVR], F16, tag="rkc", name="rkc")
            nc.gpsimd.dma_start(rkc[:], rkc_d[:, :])
            rks = cpool.tile([128, KVR], F16, tag="rks", name="rks")
            nc.gpsimd.dma_start(rks[:], rks_d[:, :])
            rqc = cpool.tile([128, CH], F16, tag="rqc", name="rqc")
            nc.gpsimd.dma_start(rqc[:], rqc_d[:, :])
            rqs = cpool.tile([128, CH], F16, tag="rqs", name="rqs")
            nc.gpsimd.dma_start(rqs[:], rqs_d[:, :])

            # qT: [d, t] grouped by kv head.  Tile j rows 0:64 = group 2j
            # (its 4 heads side by side, 512 cols each), rows 64:128 =
            # group 2j+1, so QK lhsT and rhs share a base partition.
            qT = [qT_pool.tile([128, REP * CH], F16, tag=f"qT{i}",
                               name=f"qT{i}") for i in range(NKV // 2)]
            # kT: [d, t] packed 2 kv heads per tile.
            kT = [kT_pool.tile([128, KVR], F16, tag=f"kT{i}", name=f"kT{i}")
                  for i in range(NKV // 2)]
            # vext: one tile, head kvh at pitch VP; per block 64 dims+validity
            vext = v_pool.tile([128, NKV * VP], BF16, tag="vext", name="vext")
            # a_hi/a_lo: fp8 hi/lo split of 8*aT, [p=(h,d), (r, qt, q)]
            a_hi = [a_pool.tile([128, REP, NQT, 128], F8, tag=f"ah{g}",
                                name=f"ah{g}") for g in range(NKV // 2)]
            a_lo = [a_pool.tile([128, REP, NQT, 128], F8, tag=f"al{g}",
                                name=f"al{g}") for g in range(NKV // 2)]
            # third-2 x pair tiles stay resident: they double as the
            # q-projection moving operand.
            x2h = [x2_pool.tile([128, 2, CH], F8, tag=f"x2h_{c}",
                                name=f"x2h_{c}") for c in range(NP2)]
            x2l = [x2_pool.tile([128, 2, CH], F8, tag=f"x2l_{c}",
                                name=f"x2l_{c}") for c in range(NP2)]
            # wq pair tiles: rotating pool, prefetched 2 sweeps ahead
            wq_tiles = {}

            def prefetch_wq(s):
                if s > 3 or s in wq_tiles:
                    return
                tl = []
                for c in range(NP2):
                    th = wq_pool.tile([128, 2, 512], F8, tag="wqh",
                                      name="wqh", bufs=18)
                    nc.sync.dma_start(
                        th[:], wqh_d[:, 2 * c:2 * c + 2,
                                     512 * s:512 * (s + 1)])
                    tl2 = wq_pool.tile([128, 2, 512], F8, tag="wql",
                                       name="wql", bufs=18)
                    nc.sync.dma_start(
                        tl2[:], wql_d[:, 2 * c:2 * c + 2,
                                      512 * s:512 * (s + 1)])
                    tl.append((th, tl2))
                wq_tiles[s] = tl

            # ================= KV projection =================
            # interleave weight and first-third x DMA issue so the first
            # matmul's inputs arrive within a couple of microseconds
            wkt = {}
            wvt = {}
            x0t = {}
            for c in range(NP2):
                wkt[c] = (w_pool.tile([128, 2, NKV * D], F8,
                                      tag="w8", name=f"wkh{c}", bufs=32),
                          w_pool.tile([128, 2, NKV * D], F8,
                                      tag="w8", name=f"wkl{c}", bufs=32))
                nc.sync.dma_start(wkt[c][0][:], wkh_d[:, 2 * c:2 * c + 2, :])
                nc.sync.dma_start(wkt[c][1][:], wkl_d[:, 2 * c:2 * c + 2, :])
                x0t[c] = (xkv_pool.tile([128, 2, 512], F8, tag="xkvh",
                                        name="xkvh"),
                          xkv_pool.tile([128, 2, 512], F8, tag="xkvl",
                                        name="xkvl"))
                nc.sync.dma_start(x0t[c][0][:],
                                  xh_d[:, 2 * c:2 * c + 2, 0:512])
                nc.sync.dma_start(x0t[c][1][:],
                                  xl_d[:, 2 * c:2 * c + 2, 0:512])
                wvt[c] = (w_pool.tile([128, 2, NKV * D], F8,
                                      tag="w8", name=f"wvh{c}", bufs=32),
                          w_pool.tile([128, 2, NKV * D], F8,
                                      tag="w8", name=f"wvl{c}", bufs=32))
                # wv rides the SWDGE queue so the sync queue streams
                # wk/x without head-of-line blocking (V runs after K)
                nc.gpsimd.dma_start(wvt[c][0][:],
                                    wvh_d[:, 2 * c:2 * c + 2, :])
                nc.gpsimd.dma_start(wvt[c][1][:],
                                    wvl_d[:, 2 * c:2 * c + 2, :])

            for qu in range(3):
                qs = 512 * qu
                kps = [b1(B1[m]) for m in range(4)]
                v2 = [b2("B2a"), b2("B2b")]
                vps = [v2[0][:, 0:512], v2[0][:, 512:1024],
                       v2[1][:, 0:512], v2[1][:, 512:1024]]

                def kv_rope(ms=range(4)):
                    # ACT staging: ACT is idle pre-attention, and the B1
                    # bank WAR clears at the stage read, unblocking the
                    # next third's K chain sooner
                    for m in ms:
                        _rope_write(nc, rtmp, kT[m][:, qs:qs + 512],
                                    kps[m][:], rkc[:, qs:qs + 512],
                                    rks[:, qs:qs + 512], 512,
                                    stage_dve=False)
                xt = []
                for c in range(NP2):
                    if qu == 0:
                        xh, xl = x0t[c]
                    elif qu == 2:
                        xh, xl = x2h[c], x2l[c]
                        nc.sync.dma_start(
                            xh[:], xh_d[:, 2 * c:2 * c + 2, qs:qs + 512])
                        nc.sync.dma_start(
                            xl[:], xl_d[:, 2 * c:2 * c + 2, qs:qs + 512])
                    else:
                        xh = xkv_pool.tile([128, 2, 512], F8, tag="xkvh",
                                           name="xkvh")
                        xl = xkv_pool.tile([128, 2, 512], F8, tag="xkvl",
                                           name="xkvl")
                        nc.sync.dma_start(
                            xh[:], xh_d[:, 2 * c:2 * c + 2, qs:qs + 512])
                        nc.sync.dma_start(
                            xl[:], xl_d[:, 2 * c:2 * c + 2, qs:qs + 512])
                    xt.append((xh, xl))
                    # K first (wk + x DMAs pace ahead of wv)
                    for m in range(4):
                        wsl_h = wkt[c][0][:, :, 128 * m:128 * (m + 1)]
                        wsl_l = wkt[c][1][:, :, 128 * m:128 * (m + 1)]
                        nc.tensor.matmul(kps[m][:], wsl_h, xh[:],
                                         start=(c == 0), stop=False,
                                         perf_mode=DR)
                        nc.tensor.matmul(kps[m][:], wsl_h, xl[:],
                                         start=False, stop=False,
                                         perf_mode=DR)
                        nc.tensor.matmul(kps[m][:], wsl_l, xh[:],
                                         start=False, stop=(c == NP2 - 1),
                                         perf_mode=DR)
                # ropes drain the K psums now, while V still runs, so
                # the next third's K chains never wait on the stage reads
                order = (2, 3, 0, 1) if qu == 2 else (0, 1, 2, 3)
                for m in order:
                    kv_rope(ms=(m,))
                for c in range(NP2):
                    xh, xl = xt[c]
                    for st in range(4):
                        xsl_h = xh[:, :, 128 * st:128 * (st + 1)]
                        xsl_l = xl[:, :, 128 * st:128 * (st + 1)]
                        nc.tensor.matmul(vps[st], xsl_h, wvt[c][0][:],
                                         start=(c == 0), stop=False,
                                         perf_mode=DR)
                        nc.tensor.matmul(vps[st], xsl_l, wvt[c][0][:],
                                         start=False, stop=False,
                                         perf_mode=DR)
                        nc.tensor.matmul(vps[st], xsl_h, wvt[c][1][:],
                                         start=False, stop=(c == NP2 - 1),
                                         perf_mode=DR)

                def kv_vext():
                    # vps[st] is [128 t-sub, 512 = 8 kv heads x 64 dims]
                    for st in range(4):
                        tlk = 4 * qu + st
                        dst = vext[:].rearrange(
                            "p (h b w) -> p h b w",
                            h=NKV, b=NKB)[:, :, tlk:tlk + 1, 0:D]
                        src = vps[st].rearrange(
                            "p (o h d) -> p h o d", o=1, h=NKV)
                        if st % 2 == 0:
                            nc.scalar.copy(dst, src)
                        else:
                            nc.vector.tensor_copy(dst, src)

                def kv_vext_one(st):
                    tlk = 4 * qu + st
                    dst = vext[:].rearrange(
                        "p (h b w) -> p h b w",
                        h=NKV, b=NKB)[:, :, tlk:tlk + 1, 0:D]
                    src = vps[st].rearrange(
                        "p (o h d) -> p h o d", o=1, h=NKV)
                    if st % 2 == 0:
                        nc.scalar.copy(dst, src)
                    else:
                        nc.vector.tensor_copy(dst, src)

                for st in range(4):
                    kv_vext_one(st)
                # validity columns for this third's blocks, all heads
                # (value 4096 = the fp8 product scale, so the denominator
                # matches the 2^12-scaled numerator and normalize cancels)
                t0 = 4 * qu
                nc.scalar.copy(
                    vext[:].rearrange("p (h b w) -> p h b w",
                                      h=NKV, b=NKB)[:, :, t0:t0 + 4,
                                                    D:D + 1],
                    kvv[:, t0:t0 + 4].rearrange(
                        "p (o b) -> p o b", o=1).to_broadcast(
                            (128, NKV, 4)))

            # ====== interleaved Q projection + attention ladder ======
            prefetch_wq(0)
            prefetch_wq(1)

            def q_steps(sweep):
                # emission steps of one q quarter-sweep (heads 8s..8s+7):
                # 4 m-tiles, each 24 DoubleRow matmuls on the B1d bank
                # then one rope step.  Dispensed one step per attention
                # lk-step so q matmuls plug the exp bubbles.
                steps = []
                for m4 in range(4):
                    qp = {}
                    qbank = B1[3] if (sweep > 0 or m4 % 2 == 0) else B1[2]

                    def mm_step(c, m4=m4, qp=qp, qbank=qbank):
                        if c == 0:
                            qp[0] = b1(qbank)
                        wh, wl = wq_tiles[sweep][c]
                        wsl_h = wh[:, :, 128 * m4:128 * (m4 + 1)]
                        wsl_l = wl[:, :, 128 * m4:128 * (m4 + 1)]
                        nc.tensor.matmul(qp[0][:], wsl_h, x2h[c][:],
                                         start=(c == 0), stop=False,
                                         perf_mode=DR)
                        nc.tensor.matmul(qp[0][:], wsl_h, x2l[c][:],
                                         start=False, stop=False,
                                         perf_mode=DR)
                        nc.tensor.matmul(qp[0][:], wsl_l, x2h[c][:],
                                         start=False, stop=(c == NP2 - 1),
                                         perf_mode=DR)

                    def rope_step(m4=m4, qp=qp):
                        _rope_write(
                            nc, rtmp,
                            qT[sweep][:, 512 * m4:512 * (m4 + 1)],
                            qp[0][:], rqc[:], rqs[:], CH,
                            stage_dve=(sweep > 0))

                    for c in range(NP2):
                        steps.append(lambda c=c, f=mm_step: f(c))
                    steps.append(rope_step)
                steps.append(lambda: prefetch_wq(sweep + 2))
                return steps

            def q_quarter(sweep):
                for s in q_steps(sweep):
                    s()

            def make_fill(steps):
                it = iter(steps)

                def fill():
                    s = next(it, None)
                    if s is not None:
                        s()
                return fill, it

            def attention_pair(gp, fill=None):
                kTt = kT[gp]
                qTg = qT[gp]
                pending = []
                for qt in range(NQT):
                    qv = [qTg[64 * h:64 * h + 64, :].rearrange(
                        "p (r t) -> p r t", r=REP)[
                            :, :, 128 * qt:128 * (qt + 1)]
                        for h in range(2)]
                    acc = [pv_acc(h) for h in range(2)]

                    def emit_qk(lk):
                        kb = qt + lk
                        ST = next_b2()
                        for h in range(2):
                            # per-rep matmuls so early reps' scores can
                            # start before the last q m-tile's rope lands
                            for r in range(REP):
                                nc.tensor.matmul(
                                    ST[:, 512 * h + 128 * r:
                                       512 * h + 128 * (r + 1)],
                                    kTt[64 * h:64 * h + 64,
                                        128 * kb:128 * (kb + 1)],
                                    qv[h][:, r:r + 1, :],
                                    start=(r == 0), stop=(r == REP - 1))
                        return ST

                    def pv_one(h, r, PT_sl, kb, start, stop):
                        g = 2 * gp + h
                        vsl = vext[:, VP * g + VW * kb:
                                   VP * g + VW * (kb + 1)]
                        # one accumulation group per bank: start zeroes
                        # the whole 2KB zero region, the other reps
                        # accumulate into pending-zero
                        nc.tensor.matmul(
                            acc[h][:, r:r + 1, 0:VW], PT_sl, vsl,
                            start=start, stop=stop)

                    # The lk=0 / lk=8 triangle blocks have complementary
                    # valid regions (jj>p vs jj<=p).  Per h, their scores
                    # land in the two bank-halves of ONE ST tile, are
                    # blended in place with copy_predicated, and share a
                    # single [128,512] exp - 2 half exps replace 2 full
                    # ones, and the B2a/B2b rotation cadence is unchanged.
                    for h in range(2):
                        if fill is not None:
                            fill()
                        STp = next_b2()
                        for r in range(REP):
                            nc.tensor.matmul(
                                STp[:, 128 * r:128 * (r + 1)],
                                kTt[64 * h:64 * h + 64,
                                    128 * qt:128 * (qt + 1)],
                                qv[h][:, r:r + 1, :],
                                start=(r == 0), stop=(r == REP - 1))
                        kb8 = qt + NWB - 1
                        for r in range(REP):
                            nc.tensor.matmul(
                                STp[:, 512 + 128 * r:512 + 128 * (r + 1)],
                                kTt[64 * h:64 * h + 64,
                                    128 * kb8:128 * (kb8 + 1)],
                                qv[h][:, r:r + 1, :],
                                start=(r == 0), stop=(r == REP - 1))
                        if pending:
                            pending.pop(0)()
                        nc.vector.copy_predicated(
                            STp[:, 512:1024], mwu[:], STp[:, 0:512])
                        PTp = pt_pool.tile([128, 512], BF16,
                                           tag="PTp", name="PTp", bufs=2)
                        nc.scalar.activation(
                            PTp[:], STp[:, 512:1024],
                            mybir.ActivationFunctionType.Exp)
                        P0 = pt_pool.tile([128, 512], BF16,
                                          tag="P0", name="P0", bufs=2)
                        nc.vector.tensor_mul(P0[:], PTp[:],
                                             mask_win[:, 0:512])
                        P8 = pt_pool.tile([128, 512], BF16,
                                          tag="P8", name="P8", bufs=2)
                        nc.vector.tensor_mul(P8[:], PTp[:],
                                             mask_causal[:, 0:512])
                        for r in range(REP):
                            pv_one(h, r, P0[:, 128 * r:128 * (r + 1)],
                                   qt, start=(r == 0), stop=False)
                        for r in range(REP):
                            pv_one(h, r, P8[:, 128 * r:128 * (r + 1)],
                                   kb8, start=False, stop=False)
                    # QK leads PV by one step so a PV stalled on the
                    # previous unit's normalize never blocks the next
                    # exp's scores
                    ST_next = emit_qk(1)
                    for lk in range(1, NWB - 1):
                        if fill is not None:
                            fill()
                        kb = qt + lk
                        ST = ST_next
                        if pending:
                            pending.pop(0)()
                        PT = pt_pool.tile([128, 1024], BF16,
                                          tag="PT", name="PT")
                        nc.scalar.activation(
                            PT[:], ST[:],
                            mybir.ActivationFunctionType.Exp)
                        if lk + 1 < NWB - 1:
                            ST_next = emit_qk(lk + 1)
                        for h in range(2):
                            for r in range(REP):
                                pv_one(h, r,
                                       PT[:, 512 * h + 128 * r:
                                          512 * h + 128 * (r + 1)],
                                       kb, start=False,
                                       stop=(lk == NWB - 2
                                             and r == REP - 1))
                    # ---- drain: normalize, transpose to [d,q], fp8 split
                    rcp = sm_pool.tile([128, 2 * REP], DT, tag="rcp",
                                       name="rcp")
                    for h in range(2):
                        nc.vector.reciprocal(
                            rcp[:, REP * h:REP * (h + 1)],
                            acc[h][:, :, D:D + 1].rearrange(
                                "p a b -> p (a b)"))
                    aU = sm_pool.tile([128, 2 * REP * D], F16, tag="aU",
                                      name="aU")
                    for h in range(2):
                        # one batched multiply per h: rcp broadcast along d
                        nc.vector.tensor_tensor(
                            out=aU[:, 256 * h:256 * (h + 1)].rearrange(
                                "p (a b) -> p a b", a=REP),
                            in0=acc[h][:, :, 0:D],
                            in1=rcp[:, REP * h:REP * (h + 1)].rearrange(
                                "p (a b) -> p a b", b=1).to_broadcast(
                                    (128, REP, D)),
                            op=mybir.AluOpType.mult)
                    # transposes + fp8 split are deferred into the next
                    # unit's early lk slots (they only read aU, not acc)
                    tp = b1(B1[2])

                    def tp_half(h, tp=tp, aU=aU):
                        # h-outer so each half's group closes before the
                        # next opens (one open group per bank)
                        for r in range(REP):
                            j = REP * h + r
                            nc.tensor.matmul(
                                tp[64 * h:64 * h + 64,
                                   128 * r:128 * (r + 1)],
                                aU[:, D * j:D * (j + 1)],
                                id8[:], start=(r == 0),
                                stop=(r == REP - 1))

                    def hilo(tp=tp, gp=gp, qt=qt):
                        hi_sl = a_hi[gp][:, :, qt:qt + 1, :]
                        lo_sl = a_lo[gp][:, :, qt:qt + 1, :]
                        s2 = tp[:].rearrange("p (r o q) -> p r o q",
                                             r=REP, o=1)
                        nc.vector.tensor_copy(hi_sl, s2)
                        nc.vector.tensor_sub(lo_sl, s2, hi_sl)

                    pending.extend(
                        [lambda h=h, f=tp_half: f(h) for h in range(2)]
                        + [hilo])
                for p in pending:
                    p()
                pending = []

            # ======= output projection =======
            wo_res = {}

            def load_wo(j, oc):
                th = w_pool.tile([128, 2, 512], F8, tag="w8",
                                 name="woh", bufs=32)
                tl = w_pool.tile([128, 2, 512], F8, tag="w8",
                                 name="wol", bufs=32)
                eng = nc.sync if (j + oc) % 2 == 0 else nc.gpsimd
                eng.dma_start(th[:], woh_d[:, 2 * j:2 * j + 2,
                                           512 * oc:512 * (oc + 1)])
                eng.dma_start(tl[:], wol_d[:, 2 * j:2 * j + 2,
                                           512 * oc:512 * (oc + 1)])
                wo_res[(j, oc)] = (th, tl)
                return th, tl

            def oproj_mms(outs, oc, j, tts):
                # the 3 hi/lo DoubleRow terms for contraction pair j
                wot = wo_res.get((j, oc))
                if wot is None:
                    wot = load_wo(j, oc)
                gp, ri = divmod(j, 2)
                for ap, tt in zip(outs, tts):
                    ah = a_hi[gp][:, 2 * ri:2 * ri + 2, tt:tt + 1, :]
                    al = a_lo[gp][:, 2 * ri:2 * ri + 2, tt:tt + 1, :]
                    nc.tensor.matmul(ap, ah, wot[0][:],
                                     start=(j == 0), stop=False,
                                     perf_mode=DR)
                    nc.tensor.matmul(ap, al, wot[0][:],
                                     start=False, stop=False,
                                     perf_mode=DR)
                    nc.tensor.matmul(ap, ah, wot[1][:],
                                     start=False, stop=(j == NP2 - 1),
                                     perf_mode=DR)

            def flush_ops(outs, oc, tts, dve=False):
                for ap, tt in zip(outs, tts):
                    st = ostage.tile([128, 512], F16,
                                     tag=f"stage{tt % 2}",
                                     name="stage", bufs=2)
                    # fold away the 2^12 fp8 product scale
                    if dve or tt % 2 == 0:
                        nc.vector.tensor_scalar(
                            out=st[:], in0=ap, scalar1=SPS, scalar2=None,
                            op0=mybir.AluOpType.mult)
                    else:
                        nc.scalar.activation(
                            st[:], ap,
                            mybir.ActivationFunctionType.Copy, scale=SPS)
                    deng = nc.gpsimd if tt % 2 == 0 else nc.scalar
                    deng.dma_start(
                        out_d[128 * tt:128 * (tt + 1),
                              512 * oc:512 * (oc + 1)], st[:])

            # ladder: q sweep s dispenses INTO attention pair s-1 so
            # every exp bubble gets dense q matmuls; q0 runs standalone
            q_quarter(0)
            for sweep in range(1, 4):
                fill, it = make_fill(q_steps(sweep))
                attention_pair(sweep - 1, fill=fill)
                for s in it:
                    s()

            # oc0's four t-chains dribble into attn3 on B1d, one 3-term
            # j-step per lk slot; j 6,7 (gp3 chunks) are gated on the qt
            # units that produce a_hi[3]
            fill_state = {"calls": 0, "tt": 0, "j": 0, "ap": None}

            def fill0():
                calls = fill_state["calls"]
                fill_state["calls"] += 1
                units_done = calls // 9
                # only the first slots of each unit, where the previous
                # unit's drain leaves the PE idle anyway
                if calls % 9 < 2 or calls % 9 > 5:
                    return
                tt = fill_state["tt"]
                if tt >= NQT:
                    return
                j = fill_state["j"]
                if j >= NP2:
                    flush_ops([fill_state["ap"]], 0, (tt,), dve=True)
                    fill_state["tt"] = tt + 1
                    fill_state["j"] = 0
                    fill_state["ap"] = None
                    return
                if j >= 6 and units_done <= tt:
                    return
                if fill_state["ap"] is None:
                    fill_state["ap"] = b1(B1[3])[:]
                oproj_mms([fill_state["ap"]], 0, j, (tt,))
                fill_state["j"] = j + 1

            attention_pair(3, fill=fill0)
            while fill_state["tt"] < NQT:
                fill0()
            for oc in range(1, 4):
                if oc % 2:
                    t2b = b2("B2b")
                    outs = [t2b[:, 0:512], t2b[:, 512:1024],
                            b1(B1[1])[:], b1(B1[2])[:]]
                else:
                    outs = [b1(B1[3])[:], b1(B1[0])[:]]
                    t2a = b2("B2a")
                    outs += [t2a[:, 0:512], t2a[:, 512:1024]]
                if oc < 3:
                    for j in range(NP2):
                        oproj_mms(outs, oc, j, (0, 1, 2, 3))
                    flush_ops(outs, oc, (0, 1, 2, 3))
                else:
                    # last round in two halves so the first flush
                    # overlaps the second half's matmuls
                    for j in range(NP2):
                        oproj_mms(outs[0:2], oc, j, (0, 1))
                    flush_ops(outs[0:2], oc, (0, 1))
                    for j in range(NP2):
                        oproj_mms(outs[2:4], oc, j, (2, 3))
                    flush_ops(outs[2:4], oc, (2, 3))

    nc.compile()
    return nc


# old-dim -> new-dim pair interleave for one 64-dim head:
# new dim 2j holds old dim j, new dim 2j+1 holds old dim j+32.
_P64 = np.empty(64, np.int64)
_P64[0::2] = np.arange(32)
_P64[1::2] = np.arange(32, 64)

_F8NP = ml_dtypes.float8_e4m3


def _split8(a, scale):
    """scaled hi/lo e4m3 split: a*scale = hi + lo (+ ~0.1% residual)"""
    a = np.asarray(a, np.float32) * scale
    hi = a.astype(_F8NP)
    lo = (a - hi.astype(np.float32)).astype(_F8NP)
    return hi, lo


def _pairs(a):
    """[C, N] -> [128, (C//256)*2, N] DoubleRow pair-chunk layout"""
    Cr, N = a.shape
    return np.ascontiguousarray(
        a.reshape(Cr // 256, 2, 128, N).transpose(2, 0, 1, 3).reshape(
            128, (Cr // 256) * 2, N))


def _rope_tables(t_idx, scale):
    """cos/sin tables in pair-interleaved [d, t] layout, 2-head packed.

    Row 2j and 2j+1 carry cos(theta_j); sin row 2j is negated (rotate-half
    sign in the interleaved layout).  Rows 64:128 repeat for head 2."""
    inv_freq = 1.0 / (ROPE_BASE ** (np.arange(0, D, 2, dtype=np.float64) / D))
    ang = t_idx[None, :] * inv_freq[:, None]          # [32, n]
    cos1 = np.cos(ang)
    sin1 = np.sin(ang)
    n = ang.shape[1]
    cos64 = np.empty((64, n))
    cos64[0::2] = cos1
    cos64[1::2] = cos1
    sin64 = np.empty((64, n))
    sin64[0::2] = -sin1
    sin64[1::2] = sin1
    cos64 *= scale
    sin64 *= scale
    return (np.tile(cos64, (2, 1)).astype(np.float16),
            np.tile(sin64, (2, 1)).astype(np.float16))


def _permute_wk(Wk):
    """Pair-interleave each kv head's 64 dims in Wk's columns."""
    idx = np.concatenate([64 * h + _P64 for h in range(NKV)])
    return Wk[:, idx]


def _permute_wq(Wq):
    """Pack Wq columns so psum m-tile m = (sweep, r) holds head 8*sweep+r
    in rows 0:64 and head 8*sweep+4+r in rows 64:128, pair-interleaved."""
    cols = []
    for m in range(16):
        tau, r = divmod(m, 4)
        hA = 8 * tau + r
        hB = 8 * tau + 4 + r
        cols.append(64 * hA + _P64)
        cols.append(64 * hB + _P64)
    return Wq[:, np.concatenate(cols)]


def _permute_wo(Wo):
    """Row-permute Wo to the on-chip a layout: contraction chunk-pair
    j = 2*gp + ri holds (i, p=(h,d)) -> head 4*(2gp+h) + 2*ri + i, dim d."""
    rows = np.empty(NH * D, np.int64)
    pos = 0
    for j in range(NP2):
        gp, ri = divmod(j, 2)
        for i in range(2):
            for h in range(2):
                hh = REP * (2 * gp + h) + 2 * ri + i
                rows[pos:pos + D] = 64 * hh + np.arange(D)
                pos += D
    return Wo[rows, :]


def make_in_maps(x, Wq, Wk, Wv, Wo):
    x = np.asarray(x, np.float32)
    bf16 = ml_dtypes.bfloat16
    ins = []
    i = np.arange(128)
    masks = {
        "mask_win8": np.tile((i[:, None] > i[None, :]).astype(bf16),
                             (1, 2 * REP)),
        "mask_causal8": np.tile((i[:, None] <= i[None, :]).astype(bf16),
                                (1, 2 * REP)),
    }
    ident8 = (8.0 * np.eye(128)).astype(np.float16)
    mask_win_u8 = np.tile((i[:, None] > i[None, :]).astype(np.uint8),
                          (1, REP))
    wqh, wql = _split8(_permute_wq(np.asarray(Wq)), SW)
    wkh, wkl = _split8(_permute_wk(np.asarray(Wk)), SW)
    wvh, wvl = _split8(np.asarray(Wv), SW)
    woh, wol = _split8(_permute_wo(np.asarray(Wo)), SW)
    wts = {
        "wq_hi": _pairs(wqh), "wq_lo": _pairs(wql),
        "wk_hi": _pairs(wkh), "wk_lo": _pairs(wkl),
        "wv_hi": _pairs(wvh), "wv_lo": _pairs(wvl),
        "wo_hi": _pairs(woh), "wo_lo": _pairs(wol),
    }
    for c in range(NCORE):
        b, ch = divmod(c, 4)
        r0 = CH * ch
        kv0 = r0 - WIN
        xT = np.ascontiguousarray(x[b].T)             # [C, T]
        xkv = np.zeros((C, KVR), np.float32)
        pad = max(0, -kv0)
        xkv[:, pad:] = xT[:, kv0 + pad:r0 + CH]
        xh, xl = _split8(xkv, SX)
        qc, qs = _rope_tables(np.arange(r0, r0 + CH, dtype=np.float64),
                              SCALE)
        kc, ks = _rope_tables(np.arange(kv0, r0 + CH, dtype=np.float64),
                              1.0)
        kvvalid = np.zeros((128, NKB), bf16)
        for lk in range(NKB):
            kvvalid[:, lk] = np.where(kv0 + 128 * lk + i >= 0,
                                      SX * SW, 0.0).astype(bf16)
        ins.append({
            "xkv_hi": _pairs(xh),
            "xkv_lo": _pairs(xl),
            "rope_q_cos": qc, "rope_q_sin": qs,
            "rope_k_cos": kc, "rope_k_sin": ks,
            "kvvalid": kvvalid,
            "ident8": ident8,
            "mask_win_u8": mask_win_u8,
            **wts,
            **masks,
        })
    return ins


_PROG_CACHE = {}


def get_program():
    if "nc" not in _PROG_CACHE:
        _PROG_CACHE["nc"] = build_program()
    return _PROG_CACHE["nc"]


def kernel(x, Wq, Wk, Wv, Wo):
    nc = get_program()
    ins = make_in_maps(x, Wq, Wk, Wv, Wo)
    res = run_bass_kernel_spmd(nc, ins, list(range(NCORE)))
    out = np.empty((B, T, C), np.float32)
    for c in range(NCORE):
        b, ch = divmod(c, 4)
        out[b, CH * ch:CH * (ch + 1), :] = res.results[c]["out"].astype(
            np.float32)
    return out


# revision 45
# speedup vs baseline: 1.1068x; 1.1068x over previous
"""Trainium2 Bass kernel for Mistral-style sliding-window GQA attention.

Problem: B=2, T=2048, C=2048, 32 q heads / 8 kv heads, head_dim=64,
sliding causal window 1024, RoPE, fp32.

Sharding (sequence-parallel, no cross-core communication):
  core c in 0..7 handles batch b=c//4 and contiguous 512-row chunk k=c%4.
  Each core computes q for its 512 rows, k/v for its rows plus a 1024-row
  halo (zero-padded before t=0), full attention for its rows over all 32
  heads, and the output projection for its rows.  Host gathers by
  concatenation only.

Device program (v4):
  - All four projections run as fp8e4m3 DoubleRow matmuls with a 3-term
    hi+lo error split (x_hi@W_hi + x_lo@W_hi + x_hi@W_lo; the dropped
    lo@lo term is ~0.1% relative).  Operands are pre-scaled (x*8, W*512)
    into e4m3's normal range; the 2^-12 product scale is folded into the
    rope stage copies, the v validity column (4096), and the output
    flush.  Each DoubleRow matmul contracts 256 rows at half the
    per-column cost, so projections cost 3/8 of their fp16 equivalent.
  - PV is reoriented: PT ([keys, q]) is the stationary operand (full
    128x128 array) and v_ext [keys, 65] moves, so each kv block costs 65
    columns instead of 512.  The accumulator lands in [q, d] layout with
    the softmax denominator in column 64; normalization is a
    per-partition reciprocal + tensor_scalar (no partition_broadcast),
    then a x8-identity matmul transposes back to [d, q] and the result
    is split on-chip into fp8 hi/lo for the o-projection.
  - Head dims are PAIR-INTERLEAVED on host for the RoPE rotate-half
    (adjacent-partition stream_shuffle), as in v3.  QK keeps fp16
    operands (fp8 scores fail the softmax error budget).
  - One fixed 8-bank PSUM choreography via six tags: B2a/B2b are 2-bank
    [128,1024] tiles (KV v psums, attention STs, o-proj rounds); B1a-d
    are 1-bank tiles (KV k psums, PV accumulators, transpose bank,
    q-proj bank, o-proj chains).
  - Q-projection DoubleRow steps are dispensed one per attention lk-step
    to plug the exp bubbles (ACT is the binding engine in the attention
    region); the oc0 o-proj chains dribble into the final attention pair
    the same way.  Score matmuls are emitted one step ahead of PV so a
    PV stalled on the previous unit's normalize never delays the next
    exp; per-unit transposes + fp8 splits are deferred into the next
    unit's early lk slots.
"""

import numpy as np
import ml_dtypes

import concourse.bass as bass
import concourse.mybir as mybir
import concourse.tile as tile
from concourse import bacc
from concourse.bass_utils import run_bass_kernel_spmd

B, T, C = 2, 2048, 2048
NH, NKV, D = 32, 8, 64
REP = NH // NKV
WIN = 1024
CH = 512          # q rows per core
KVR = CH + WIN    # kv rows per core (with halo)
NCORE = 8
DT = mybir.dt.float32
F16 = mybir.dt.float16
BF16 = mybir.dt.bfloat16
F8 = mybir.dt.float8e4
SCALE = 1.0 / np.sqrt(np.float32(D))
ROPE_BASE = 10000.0

NP2 = C // 256    # 8 contraction pair-chunks
NQT = CH // 128   # 4 q tiles per chunk
NKB = KVR // 128  # 12 kv blocks per core
NWB = 9           # kv blocks in the window of one q tile
VW = 65           # v_ext width per kv block (64 dims + validity column)
VP = NKB * VW     # per-head v_ext pitch (780)

SX = 8.0          # x pre-scale into e4m3 range
SW = 512.0        # weight pre-scale
SPS = 1.0 / (SX * SW)   # 2^-12 psum descale

# Head dims are PAIR-INTERLEAVED on host (new dim 2j = old j, 2j+1 = old
# j+32), so RoPE rotate-half is an adjacent-partition swap - expressible as
# a stream_shuffle (which permutes within 32-partition quadrants only).
_PAIR_SHUF = [i ^ 1 for i in range(32)]

DR = mybir.MatmulPerfMode.DoubleRow


def _rope_write(nc, pool, out_ap, ps, cosw, ssinw, n, stage_dve=False):
    """out = ps*cos + rot_half(ps)*sin on a [128, n] 2-head-packed tile.

    ps is a PSUM f32 tile (carrying the 2^12 fp8 product scale; the
    tables carry 2^-12); one copy stages it to fp16 SBUF (ACT by
    default, DVE when the ACT queue is exp-loaded), then all multiplies/
    adds run on DVE in fp16.  ssinw rows carry the rotate-half signs
    (rows 0-31/64-95 negated) and any folded scale; cosw carries the same
    scale.  out_ap receives the fp16 result.
    """
    rs = pool.tile([128, n], F16, tag="rope_rs", name="rope_rs")
    # the stage copy also folds away the 2^12 fp8 product scale (folding
    # it into the fp16 tables instead would push them into denormals)
    if stage_dve:
        nc.vector.tensor_scalar(
            out=rs[:], in0=ps[:], scalar1=SPS, scalar2=None,
            op0=mybir.AluOpType.mult)
    else:
        nc.scalar.activation(rs[:], ps[:],
                             mybir.ActivationFunctionType.Copy, scale=SPS)
    # rotate-half = adjacent-partition swap in the pair-interleaved layout
    sw = pool.tile([128, n], F16, tag="rope_sw", name="rope_sw")
    nc.vector.stream_shuffle(sw[:], rs[:], _PAIR_SHUF)
    t2 = pool.tile([128, n], F16, tag="rope_t2", name="rope_t2")
    nc.vector.tensor_mul(t2[:], sw[:], ssinw[:])
    t1 = pool.tile([128, n], F16, tag="rope_t1", name="rope_t1")
    nc.vector.tensor_mul(t1[:], rs[:], cosw[:])
    nc.vector.tensor_add(out_ap, t1[:], t2[:])


def build_program():
    nc = bacc.Bacc("TRN2", target_bir_lowering=False, debug=False,
                   num_devices=NCORE)

    xh_d = nc.dram_tensor("xkv_hi", [128, NP2 * 2, KVR], F8,
                          kind="ExternalInput")
    xl_d = nc.dram_tensor("xkv_lo", [128, NP2 * 2, KVR], F8,
                          kind="ExternalInput")
    wqh_d = nc.dram_tensor("wq_hi", [128, NP2 * 2, NH * D], F8,
                           kind="ExternalInput")
    wql_d = nc.dram_tensor("wq_lo", [128, NP2 * 2, NH * D], F8,
                           kind="ExternalInput")
    wkh_d = nc.dram_tensor("wk_hi", [128, NP2 * 2, NKV * D], F8,
                           kind="ExternalInput")
    wkl_d = nc.dram_tensor("wk_lo", [128, NP2 * 2, NKV * D], F8,
                           kind="ExternalInput")
    wvh_d = nc.dram_tensor("wv_hi", [128, NP2 * 2, NKV * D], F8,
                           kind="ExternalInput")
    wvl_d = nc.dram_tensor("wv_lo", [128, NP2 * 2, NKV * D], F8,
                           kind="ExternalInput")
    woh_d = nc.dram_tensor("wo_hi", [128, NP2 * 2, C], F8,
                           kind="ExternalInput")
    wol_d = nc.dram_tensor("wo_lo", [128, NP2 * 2, C], F8,
                           kind="ExternalInput")
    rqc_d = nc.dram_tensor("rope_q_cos", [128, CH], F16, kind="ExternalInput")
    rqs_d = nc.dram_tensor("rope_q_sin", [128, CH], F16, kind="ExternalInput")
    rkc_d = nc.dram_tensor("rope_k_cos", [128, KVR], F16, kind="ExternalInput")
    rks_d = nc.dram_tensor("rope_k_sin", [128, KVR], F16, kind="ExternalInput")
    kvv_d = nc.dram_tensor("kvvalid", [128, NKB], BF16, kind="ExternalInput")
    mw_d = nc.dram_tensor("mask_win8", [128, 1024], BF16, kind="ExternalInput")
    mc_d = nc.dram_tensor("mask_causal8", [128, 1024], BF16,
                          kind="ExternalInput")
    id8_d = nc.dram_tensor("ident8", [128, 128], F16, kind="ExternalInput")
    out_d = nc.dram_tensor("out", [CH, C], F16, kind="ExternalOutput")

    with tile.TileContext(nc) as tc:
        with (
            tc.tile_pool(name="const", bufs=1) as cpool,
            tc.tile_pool(name="qT", bufs=1) as qT_pool,
            tc.tile_pool(name="kT", bufs=1) as kT_pool,
            tc.tile_pool(name="vext", bufs=1) as v_pool,
            tc.tile_pool(name="x2", bufs=1) as x2_pool,
            tc.tile_pool(name="ahi", bufs=1) as a_pool,
            tc.tile_pool(name="wq_s", bufs=1) as wq_pool,
            tc.tile_pool(name="xkv_s", bufs=12) as xkv_pool,
            tc.tile_pool(name="w8", bufs=32) as w_pool,
            tc.tile_pool(name="rope_tmp", bufs=2) as rtmp,
            tc.tile_pool(name="pt", bufs=4) as pt_pool,
            tc.tile_pool(name="att_small", bufs=3) as sm_pool,
            tc.tile_pool(name="ostage", bufs=2) as ostage,
            tc.tile_pool(name="psum", bufs=1, space="PSUM") as psum,
        ):
            # PSUM is hand-choreographed on 8 banks via six fixed tags (no
            # pool transitions, so no pipeline barriers anywhere).
            def b2(tag):
                return psum.tile([128, 1024], DT, tag=tag, name=tag)

            def b1(tag):
                return psum.tile([128, 512], DT, tag=tag, name=tag)

            B1 = ["B1a", "B1b", "B1c", "B1d"]
            st_ctr = [0]

            def next_b2():
                t = ["B2a", "B2b"][st_ctr[0] % 2]
                st_ctr[0] += 1
                return b2(t)

            def pv_acc(h):
                # [q, (r, 128)] accumulator inside one B1 bank: 128-word
                # pitch so each rep's accumulation group owns its own
                # 512-byte PSUM zero region (65-word pitch would put two
                # open groups in one zero region, which start=True zeroes)
                t = b1(B1[h])
                return t[:].rearrange("p (r w) -> p r w", r=REP)

            # ---- constants (small, persistent; SWDGE queue) ----
            mask_win = cpool.tile([128, 1024], BF16, tag="mw", name="mask_win")
            nc.gpsimd.dma_start(mask_win[:], mw_d[:, :])
            mask_causal = cpool.tile([128, 1024], BF16, tag="mc",
                                     name="mask_causal")
            nc.gpsimd.dma_start(mask_causal[:], mc_d[:, :])
            kvv = cpool.tile([128, NKB], BF16, tag="kvv", name="kvv")
            nc.gpsimd.dma_start(kvv[:], kvv_d[:, :])
            id8 = cpool.tile([128, 128], F16, tag="id8", name="id8")
            nc.gpsimd.dma_start(id8[:], id8_d[:, :])
            rkc = cpool.tile([128, KVR], F16, tag="rkc", name="rkc")
            nc.gpsimd.dma_start(rkc[:], rkc_d[:, :])
            rks = cpool.tile([128, KVR], F16, tag="rks", name="rks")
            nc.gpsimd.dma_start(rks[:], rks_d[:, :])
            rqc = cpool.tile([128, CH], F16, tag="rqc", name="rqc")
            nc.gpsimd.dma_start(rqc[:], rqc_d[:, :])
            rqs = cpool.tile([128, CH], F16, tag="rqs", name="rqs")
            nc.gpsimd.dma_start(rqs[:], rqs_d[:, :])

            # qT: [d, t] grouped by kv head.  Tile j rows 0:64 = group 2j
            # (its 4 heads side by side, 512 cols each), rows 64:128 =
            # group 2j+1, so QK lhsT and rhs share a base partition.
            qT = [qT_pool.tile([128, REP * CH], F16, tag=f"qT{i}",
                               name=f"qT{i}") for i in range(NKV // 2)]
            # kT: [d, t] packed 2 kv heads per tile.
            kT = [kT_pool.tile([128, KVR], F16, tag=f"kT{i}", name=f"kT{i}")
                  for i in range(NKV // 2)]
            # vext: one tile, head kvh at pitch VP; per block 64 dims+validity
            vext = v_pool.tile([128, NKV * VP], BF16, tag="vext", name="vext")
            # a_hi/a_lo: fp8 hi/lo split of 8*aT, [p=(h,d), (r, qt, q)]
            a_hi = [a_pool.tile([128, REP, NQT, 128], F8, tag=f"ah{g}",
                                name=f"ah{g}") for g in range(NKV // 2)]
            a_lo = [a_pool.tile([128, REP, NQT, 128], F8, tag=f"al{g}",
                                name=f"al{g}") for g in range(NKV // 2)]
            # third-2 x pair tiles stay resident: they double as the
            # q-projection moving operand.
            x2h = [x2_pool.tile([128, 2, CH], F8, tag=f"x2h_{c}",
                                name=f"x2h_{c}") for c in range(NP2)]
            x2l = [x2_pool.tile([128, 2, CH], F8, tag=f"x2l_{c}",
                                name=f"x2l_{c}") for c in range(NP2)]
            # wq pair tiles: rotating pool, prefetched 2 sweeps ahead
            wq_tiles = {}

            def prefetch_wq(s):
                if s > 3 or s in wq_tiles:
                    return
                tl = []
                for c in range(NP2):
                    th = wq_pool.tile([128, 2, 512], F8, tag="wqh",
                                      name="wqh", bufs=18)
                    nc.sync.dma_start(
                        th[:], wqh_d[:, 2 * c:2 * c + 2,
                                     512 * s:512 * (s + 1)])
                    tl2 = wq_pool.tile([128, 2, 512], F8, tag="wql",
                                       name="wql", bufs=18)
                    nc.sync.dma_start(
                        tl2[:], wql_d[:, 2 * c:2 * c + 2,
                                      512 * s:512 * (s + 1)])
                    tl.append((th, tl2))
                wq_tiles[s] = tl

            # ================= KV projection =================
            # interleave weight and first-third x DMA issue so the first
            # matmul's inputs arrive within a couple of microseconds
            wkt = {}
            wvt = {}
            x0t = {}
            for c in range(NP2):
                wkt[c] = (w_pool.tile([128, 2, NKV * D], F8,
                                      tag="w8", name=f"wkh{c}", bufs=32),
                          w_pool.tile([128, 2, NKV * D], F8,
                                      tag="w8", name=f"wkl{c}", bufs=32))
                nc.sync.dma_start(wkt[c][0][:], wkh_d[:, 2 * c:2 * c + 2, :])
                nc.sync.dma_start(wkt[c][1][:], wkl_d[:, 2 * c:2 * c + 2, :])
                x0t[c] = (xkv_pool.tile([128, 2, 512], F8, tag="xkvh",
                                        name="xkvh"),
                          xkv_pool.tile([128, 2, 512], F8, tag="xkvl",
                                        name="xkvl"))
                nc.sync.dma_start(x0t[c][0][:],
                                  xh_d[:, 2 * c:2 * c + 2, 0:512])
                nc.sync.dma_start(x0t[c][1][:],
                                  xl_d[:, 2 * c:2 * c + 2, 0:512])
                wvt[c] = (w_pool.tile([128, 2, NKV * D], F8,
                                      tag="w8", name=f"wvh{c}", bufs=32),
                          w_pool.tile([128, 2, NKV * D], F8,
                                      tag="w8", name=f"wvl{c}", bufs=32))
                # wv rides the SWDGE queue so the sync queue streams
                # wk/x without head-of-line blocking (V runs after K)
                nc.gpsimd.dma_start(wvt[c][0][:],
                                    wvh_d[:, 2 * c:2 * c + 2, :])
                nc.gpsimd.dma_start(wvt[c][1][:],
                                    wvl_d[:, 2 * c:2 * c + 2, :])

            for qu in range(3):
                qs = 512 * qu
                kps = [b1(B1[m]) for m in range(4)]
                v2 = [b2("B2a"), b2("B2b")]
                vps = [v2[0][:, 0:512], v2[0][:, 512:1024],
                       v2[1][:, 0:512], v2[1][:, 512:1024]]

                def kv_rope(ms=range(4)):
                    # ACT staging: ACT is idle pre-attention, and the B1
                    # bank WAR clears at the stage read, unblocking the
                    # next third's K chain sooner
                    for m in ms:
                        _rope_write(nc, rtmp, kT[m][:, qs:qs + 512],
                                    kps[m][:], rkc[:, qs:qs + 512],
                                    rks[:, qs:qs + 512], 512,
                                    stage_dve=False)
                xt = []
                for c in range(NP2):
                    if qu == 0:
                        xh, xl = x0t[c]
                    elif qu == 2:
                        xh, xl = x2h[c], x2l[c]
                        nc.sync.dma_start(
                            xh[:], xh_d[:, 2 * c:2 * c + 2, qs:qs + 512])
                        nc.sync.dma_start(
                            xl[:], xl_d[:, 2 * c:2 * c + 2, qs:qs + 512])
                    else:
                        xh = xkv_pool.tile([128, 2, 512], F8, tag="xkvh",
                                           name="xkvh")
                        xl = xkv_pool.tile([128, 2, 512], F8, tag="xkvl",
                                           name="xkvl")
                        nc.sync.dma_start(
                            xh[:], xh_d[:, 2 * c:2 * c + 2, qs:qs + 512])
                        nc.sync.dma_start(
                            xl[:], xl_d[:, 2 * c:2 * c + 2, qs:qs + 512])
                    xt.append((xh, xl))
                    # K first (wk + x DMAs pace ahead of wv)
                    for m in range(4):
                        wsl_h = wkt[c][0][:, :, 128 * m:128 * (m + 1)]
                        wsl_l = wkt[c][1][:, :, 128 * m:128 * (m + 1)]
                        nc.tensor.matmul(kps[m][:], wsl_h, xh[:],
                                         start=(c == 0), stop=False,
                                         perf_mode=DR)
                        nc.tensor.matmul(kps[m][:], wsl_h, xl[:],
                                         start=False, stop=False,
                                         perf_mode=DR)
                        nc.tensor.matmul(kps[m][:], wsl_l, xh[:],
                                         start=False, stop=(c == NP2 - 1),
                                         perf_mode=DR)
                # ropes drain the K psums now, while V still runs, so
                # the next third's K chains never wait on the stage reads
                order = (2, 3, 0, 1) if qu == 2 else (0, 1, 2, 3)
                for m in order:
                    kv_rope(ms=(m,))
                for c in range(NP2):
                    xh, xl = xt[c]
                    for st in range(4):
                        xsl_h = xh[:, :, 128 * st:128 * (st + 1)]
                        xsl_l = xl[:, :, 128 * st:128 * (st + 1)]
                        nc.tensor.matmul(vps[st], xsl_h, wvt[c][0][:],
                                         start=(c == 0), stop=False,
                                         perf_mode=DR)
                        nc.tensor.matmul(vps[st], xsl_l, wvt[c][0][:],
                                         start=False, stop=False,
                                         perf_mode=DR)
                        nc.tensor.matmul(vps[st], xsl_h, wvt[c][1][:],
                                         start=False, stop=(c == NP2 - 1),
                                         perf_mode=DR)

                def kv_vext():
                    # vps[st] is [128 t-sub, 512 = 8 kv heads x 64 dims]
                    for st in range(4):
                        tlk = 4 * qu + st
                        dst = vext[:].rearrange(
                            "p (h b w) -> p h b w",
                            h=NKV, b=NKB)[:, :, tlk:tlk + 1, 0:D]
                        src = vps[st].rearrange(
                            "p (o h d) -> p h o d", o=1, h=NKV)
                        if st % 2 == 0:
                            nc.scalar.copy(dst, src)
                        else:
                            nc.vector.tensor_copy(dst, src)

                def kv_vext_one(st):
                    tlk = 4 * qu + st
                    dst = vext[:].rearrange(
                        "p (h b w) -> p h b w",
                        h=NKV, b=NKB)[:, :, tlk:tlk + 1, 0:D]
                    src = vps[st].rearrange(
                        "p (o h d) -> p h o d", o=1, h=NKV)
                    if st % 2 == 0:
                        nc.scalar.copy(dst, src)
                    else:
                        nc.vector.tensor_copy(dst, src)

                for st in range(4):
                    kv_vext_one(st)
                # validity columns for this third's blocks, all heads
                # (value 4096 = the fp8 product scale, so the denominator
                # matches the 2^12-scaled numerator and normalize cancels)
                t0 = 4 * qu
                nc.scalar.copy(
                    vext[:].rearrange("p (h b w) -> p h b w",
                                      h=NKV, b=NKB)[:, :, t0:t0 + 4,
                                                    D:D + 1],
                    kvv[:, t0:t0 + 4].rearrange(
                        "p (o b) -> p o b", o=1).to_broadcast(
                            (128, NKV, 4)))

            # ====== interleaved Q projection + attention ladder ======
            prefetch_wq(0)
            prefetch_wq(1)

            def q_steps(sweep):
                # emission steps of one q quarter-sweep (heads 8s..8s+7):
                # 4 m-tiles, each 24 DoubleRow matmuls on the B1d bank
                # then one rope step.  Dispensed one step per attention
                # lk-step so q matmuls plug the exp bubbles.
                steps = []
                for m4 in range(4):
                    qp = {}
                    qbank = B1[3] if (sweep > 0 or m4 % 2 == 0) else B1[2]

                    def mm_step(c, m4=m4, qp=qp, qbank=qbank):
                        if c == 0:
                            qp[0] = b1(qbank)
                        wh, wl = wq_tiles[sweep][c]
                        wsl_h = wh[:, :, 128 * m4:128 * (m4 + 1)]
                        wsl_l = wl[:, :, 128 * m4:128 * (m4 + 1)]
                        nc.tensor.matmul(qp[0][:], wsl_h, x2h[c][:],
                                         start=(c == 0), stop=False,
                                         perf_mode=DR)
                        nc.tensor.matmul(qp[0][:], wsl_h, x2l[c][:],
                                         start=False, stop=False,
                                         perf_mode=DR)
                        nc.tensor.matmul(qp[0][:], wsl_l, x2h[c][:],
                                         start=False, stop=(c == NP2 - 1),
                                         perf_mode=DR)

                    def rope_step(m4=m4, qp=qp):
                        _rope_write(
                            nc, rtmp,
                            qT[sweep][:, 512 * m4:512 * (m4 + 1)],
                            qp[0][:], rqc[:], rqs[:], CH,
                            stage_dve=(sweep > 0))

                    for c in range(NP2):
                        steps.append(lambda c=c, f=mm_step: f(c))
                    steps.append(rope_step)
                steps.append(lambda: prefetch_wq(sweep + 2))
                return steps

            def q_quarter(sweep):
                for s in q_steps(sweep):
                    s()

            def make_fill(steps):
                it = iter(steps)

                def fill():
                    s = next(it, None)
                    if s is not None:
                        s()
                return fill, it

            def attention_pair(gp, fill=None):
                kTt = kT[gp]
                qTg = qT[gp]
                pending = []
                for qt in range(NQT):
                    qv = [qTg[64 * h:64 * h + 64, :].rearrange(
                        "p (r t) -> p r t", r=REP)[
                            :, :, 128 * qt:128 * (qt + 1)]
                        for h in range(2)]
                    acc = [pv_acc(h) for h in range(2)]

                    def emit_qk(lk):
                        kb = qt + lk
                        ST = next_b2()
                        for h in range(2):
                            # per-rep matmuls so early reps' scores can
                            # start before the last q m-tile's rope lands
                            for r in range(REP):
                                nc.tensor.matmul(
                                    ST[:, 512 * h + 128 * r:
                                       512 * h + 128 * (r + 1)],
                                    kTt[64 * h:64 * h + 64,
                                        128 * kb:128 * (kb + 1)],
                                    qv[h][:, r:r + 1, :],
                                    start=(r == 0), stop=(r == REP - 1))
                        return ST

                    # QK leads PV by one step so a PV stalled on the
                    # previous unit's normalize never blocks the next
                    # exp's scores
                    ST_next = emit_qk(0)
                    for lk in range(NWB):
                        if fill is not None:
                            fill()
                        kb = qt + lk
                        ST = ST_next
                        if pending and lk >= 1:
                            pending.pop(0)()
                        PT = pt_pool.tile([128, 1024], BF16,
                                          tag="PT", name="PT")
                        nc.scalar.activation(
                            PT[:], ST[:],
                            mybir.ActivationFunctionType.Exp)
                        if lk == 0:
                            nc.vector.tensor_mul(PT[:], PT[:],
                                                 mask_win[:])
                        elif lk == NWB - 1:
                            nc.vector.tensor_mul(PT[:], PT[:],
                                                 mask_causal[:])
                        if lk + 1 < NWB:
                            ST_next = emit_qk(lk + 1)
                        for h in range(2):
                            g = 2 * gp + h
                            vsl = vext[:, VP * g + VW * kb:
                                       VP * g + VW * (kb + 1)]
                            for r in range(REP):
                                # one accumulation group per bank: start
                                # zeroes the whole 2KB zero region, the
                                # other reps accumulate into pending-zero
                                nc.tensor.matmul(
                                    acc[h][:, r:r + 1, 0:VW],
                                    PT[:, 512 * h + 128 * r:
                                       512 * h + 128 * (r + 1)],
                                    vsl, start=(lk == 0 and r == 0),
                                    stop=(lk == NWB - 1 and r == REP - 1))
                    # ---- drain: normalize, transpose to [d,q], fp8 split
                    rcp = sm_pool.tile([128, 2 * REP], DT, tag="rcp",
                                       name="rcp")
                    for h in range(2):
                        nc.vector.reciprocal(
                            rcp[:, REP * h:REP * (h + 1)],
                            acc[h][:, :, D:D + 1].rearrange(
                                "p a b -> p (a b)"))
                    aU = sm_pool.tile([128, 2 * REP * D], F16, tag="aU",
                                      name="aU")
                    for h in range(2):
                        # one batched multiply per h: rcp broadcast along d
                        nc.vector.tensor_tensor(
                            out=aU[:, 256 * h:256 * (h + 1)].rearrange(
                                "p (a b) -> p a b", a=REP),
                            in0=acc[h][:, :, 0:D],
                            in1=rcp[:, REP * h:REP * (h + 1)].rearrange(
                                "p (a b) -> p a b", b=1).to_broadcast(
                                    (128, REP, D)),
                            op=mybir.AluOpType.mult)
                    # transposes + fp8 split are deferred into the next
                    # unit's early lk slots (they only read aU, not acc)
                    tp = b1(B1[2])

                    def tp_half(h, tp=tp, aU=aU):
                        # h-outer so each half's group closes before the
                        # next opens (one open group per bank)
                        for r in range(REP):
                            j = REP * h + r
                            nc.tensor.matmul(
                                tp[64 * h:64 * h + 64,
                                   128 * r:128 * (r + 1)],
                                aU[:, D * j:D * (j + 1)],
                                id8[:], start=(r == 0),
                                stop=(r == REP - 1))

                    def hilo(tp=tp, gp=gp, qt=qt):
                        hi_sl = a_hi[gp][:, :, qt:qt + 1, :]
                        lo_sl = a_lo[gp][:, :, qt:qt + 1, :]
                        s2 = tp[:].rearrange("p (r o q) -> p r o q",
                                             r=REP, o=1)
                        nc.vector.tensor_copy(hi_sl, s2)
                        nc.vector.tensor_sub(lo_sl, s2, hi_sl)

                    pending.extend(
                        [lambda h=h, f=tp_half: f(h) for h in range(2)]
                        + [hilo])
                for p in pending:
                    p()
                pending = []

            # ======= output projection =======
            wo_res = {}

            def load_wo(j, oc):
                th = w_pool.tile([128, 2, 512], F8, tag="w8",
                                 name="woh", bufs=32)
                tl = w_pool.tile([128, 2, 512], F8, tag="w8",
                                 name="wol", bufs=32)
                eng = nc.sync if (j + oc) % 2 == 0 else nc.gpsimd
                eng.dma_start(th[:], woh_d[:, 2 * j:2 * j + 2,
                                           512 * oc:512 * (oc + 1)])
                eng.dma_start(tl[:], wol_d[:, 2 * j:2 * j + 2,
                                           512 * oc:512 * (oc + 1)])
                wo_res[(j, oc)] = (th, tl)
                return th, tl

            def oproj_mms(outs, oc, j, tts):
                # the 3 hi/lo DoubleRow terms for contraction pair j
                wot = wo_res.get((j, oc))
                if wot is None:
                    wot = load_wo(j, oc)
                gp, ri = divmod(j, 2)
                for ap, tt in zip(outs, tts):
                    ah = a_hi[gp][:, 2 * ri:2 * ri + 2, tt:tt + 1, :]
                    al = a_lo[gp][:, 2 * ri:2 * ri + 2, tt:tt + 1, :]
                    nc.tensor.matmul(ap, ah, wot[0][:],
                                     start=(j == 0), stop=False,
                                     perf_mode=DR)
                    nc.tensor.matmul(ap, al, wot[0][:],
                                     start=False, stop=False,
                                     perf_mode=DR)
                    nc.tensor.matmul(ap, ah, wot[1][:],
                                     start=False, stop=(j == NP2 - 1),
                                     perf_mode=DR)

            def flush_ops(outs, oc, tts, dve=False):
                for ap, tt in zip(outs, tts):
                    st = ostage.tile([128, 512], F16,
                                     tag=f"stage{tt % 2}",
                                     name="stage", bufs=2)
                    # fold away the 2^12 fp8 product scale
                    if dve or tt % 2 == 0:
                        nc.vector.tensor_scalar(
                            out=st[:], in0=ap, scalar1=SPS, scalar2=None,
                            op0=mybir.AluOpType.mult)
                    else:
                        nc.scalar.activation(
                            st[:], ap,
                            mybir.ActivationFunctionType.Copy, scale=SPS)
                    deng = nc.gpsimd if tt % 2 == 0 else nc.scalar
                    deng.dma_start(
                        out_d[128 * tt:128 * (tt + 1),
                              512 * oc:512 * (oc + 1)], st[:])

            # ladder: q sweep s dispenses INTO attention pair s-1 so
            # every exp bubble gets dense q matmuls; q0 runs standalone
            q_quarter(0)
            for sweep in range(1, 4):
                fill, it = make_fill(q_steps(sweep))
                attention_pair(sweep - 1, fill=fill)
                for s in it:
                    s()

            # oc0's four t-chains dribble into attn3 on B1d, one 3-term
            # j-step per lk slot; j 6,7 (gp3 chunks) are gated on the qt
            # units that produce a_hi[3]
            fill_state = {"calls": 0, "tt": 0, "j": 0, "ap": None}

            def fill0():
                calls = fill_state["calls"]
                fill_state["calls"] += 1
                units_done = calls // 9
                # only the first slots of each unit, where the previous
                # unit's drain leaves the PE idle anyway
                if calls % 9 < 2 or calls % 9 > 5:
                    return
                tt = fill_state["tt"]
                if tt >= NQT:
                    return
                j = fill_state["j"]
                if j >= NP2:
                    flush_ops([fill_state["ap"]], 0, (tt,), dve=True)
                    fill_state["tt"] = tt + 1
                    fill_state["j"] = 0
                    fill_state["ap"] = None
                    return
                if j >= 6 and units_done <= tt:
                    return
                if fill_state["ap"] is None:
                    fill_state["ap"] = b1(B1[3])[:]
                oproj_mms([fill_state["ap"]], 0, j, (tt,))
                fill_state["j"] = j + 1

            attention_pair(3, fill=fill0)
            while fill_state["tt"] < NQT:
                fill0()
            for oc in range(1, 4):
                if oc % 2:
                    t2b = b2("B2b")
                    outs = [t2b[:, 0:512], t2b[:, 512:1024],
                            b1(B1[1])[:], b1(B1[2])[:]]
                else:
                    outs = [b1(B1[3])[:], b1(B1[0])[:]]
                    t2a = b2("B2a")
                    outs += [t2a[:, 0:512], t2a[:, 512:1024]]
                if oc < 3:
                    for j in range(NP2):
                        oproj_mms(outs, oc, j, (0, 1, 2, 3))
                    flush_ops(outs, oc, (0, 1, 2, 3))
                else:
                    # last round in two halves so the first flush
                    # overlaps the second half's matmuls
                    for j in range(NP2):
                        oproj_mms(outs[0:2], oc, j, (0, 1))
                    flush_ops(outs[0:2], oc, (0, 1))
                    for j in range(NP2):
                        oproj_mms(outs[2:4], oc, j, (2, 3))
                    flush_ops(outs[2:4], oc, (2, 3))

    nc.compile()
    return nc


# old-dim -> new-dim pair interleave for one 64-dim head:
# new dim 2j holds old dim j, new dim 2j+1 holds old dim j+32.
_P64 = np.empty(64, np.int64)
_P64[0::2] = np.arange(32)
_P64[1::2] = np.arange(32, 64)

_F8NP = ml_dtypes.float8_e4m3


def _split8(a, scale):
    """scaled hi/lo e4m3 split: a*scale = hi + lo (+ ~0.1% residual)"""
    a = np.asarray(a, np.float32) * scale
    hi = a.astype(_F8NP)
    lo = (a - hi.astype(np.float32)).astype(_F8NP)
    return hi, lo


def _pairs(a):
    """[C, N] -> [128, (C//256)*2, N] DoubleRow pair-chunk layout"""
    Cr, N = a.shape
    return np.ascontiguousarray(
        a.reshape(Cr // 256, 2, 128, N).transpose(2, 0, 1, 3).reshape(
            128, (Cr // 256) * 2, N))


def _rope_tables(t_idx, scale):
    """cos/sin tables in pair-interleaved [d, t] layout, 2-head packed.

    Row 2j and 2j+1 carry cos(theta_j); sin row 2j is negated (rotate-half
    sign in the interleaved layout).  Rows 64:128 repeat for head 2."""
    inv_freq = 1.0 / (ROPE_BASE ** (np.arange(0, D, 2, dtype=np.float64) / D))
    ang = t_idx[None, :] * inv_freq[:, None]          # [32, n]
    cos1 = np.cos(ang)
    sin1 = np.sin(ang)
    n = ang.shape[1]
    cos64 = np.empty((64, n))
    cos64[0::2] = cos1
    cos64[1::2] = cos1
    sin64 = np.empty((64, n))
    sin64[0::2] = -sin1
    sin64[1::2] = sin1
    cos64 *= scale
    sin64 *= scale
    return (np.tile(cos64, (2, 1)).astype(np.float16),
            np.tile(sin64, (2, 1)).astype(np.float16))


def _permute_wk(Wk):
    """Pair-interleave each kv head's 64 dims in Wk's columns."""
    idx = np.concatenate([64 * h + _P64 for h in range(NKV)])
    return Wk[:, idx]


def _permute_wq(Wq):
    """Pack Wq columns so psum m-tile m = (sweep, r) holds head 8*sweep+r
    in rows 0:64 and head 8*sweep+4+r in rows 64:128, pair-interleaved."""
    cols = []
    for m in range(16):
        tau, r = divmod(m, 4)
        hA = 8 * tau + r
        hB = 8 * tau + 4 + r
        cols.append(64 * hA + _P64)
        cols.append(64 * hB + _P64)
    return Wq[:, np.concatenate(cols)]


def _permute_wo(Wo):
    """Row-permute Wo to the on-chip a layout: contraction chunk-pair
    j = 2*gp + ri holds (i, p=(h,d)) -> head 4*(2gp+h) + 2*ri + i, dim d."""
    rows = np.empty(NH * D, np.int64)
    pos = 0
    for j in range(NP2):
        gp, ri = divmod(j, 2)
        for i in range(2):
            for h in range(2):
                hh = REP * (2 * gp + h) + 2 * ri + i
                rows[pos:pos + D] = 64 * hh + np.arange(D)
                pos += D
    return Wo[rows, :]


def make_in_maps(x, Wq, Wk, Wv, Wo):
    x = np.asarray(x, np.float32)
    bf16 = ml_dtypes.bfloat16
    ins = []
    i = np.arange(128)
    masks = {
        "mask_win8": np.tile((i[:, None] > i[None, :]).astype(bf16),
                             (1, 2 * REP)),
        "mask_causal8": np.tile((i[:, None] <= i[None, :]).astype(bf16),
                                (1, 2 * REP)),
    }
    ident8 = (8.0 * np.eye(128)).astype(np.float16)
    wqh, wql = _split8(_permute_wq(np.asarray(Wq)), SW)
    wkh, wkl = _split8(_permute_wk(np.asarray(Wk)), SW)
    wvh, wvl = _split8(np.asarray(Wv), SW)
    woh, wol = _split8(_permute_wo(np.asarray(Wo)), SW)
    wts = {
        "wq_hi": _pairs(wqh), "wq_lo": _pairs(wql),
        "wk_hi": _pairs(wkh), "wk_lo": _pairs(wkl),
        "wv_hi": _pairs(wvh), "wv_lo": _pairs(wvl),
        "wo_hi": _pairs(woh), "wo_lo": _pairs(wol),
    }
    for c in range(NCORE):
        b, ch = divmod(c, 4)
        r0 = CH * ch
        kv0 = r0 - WIN
        xT = np.ascontiguousarray(x[b].T)             # [C, T]
        xkv = np.zeros((C, KVR), np.float32)
        pad = max(0, -kv0)
        xkv[:, pad:] = xT[:, kv0 + pad:r0 + CH]
        xh, xl = _split8(xkv, SX)
        qc, qs = _rope_tables(np.arange(r0, r0 + CH, dtype=np.float64),
                              SCALE)
        kc, ks = _rope_tables(np.arange(kv0, r0 + CH, dtype=np.float64),
                              1.0)
        kvvalid = np.zeros((128, NKB), bf16)
        for lk in range(NKB):
            kvvalid[:, lk] = np.where(kv0 + 128 * lk + i >= 0,
                                      SX * SW, 0.0).astype(bf16)
        ins.append({
            "xkv_hi": _pairs(xh),
            "xkv_lo": _pairs(xl),
            "rope_q_cos": qc, "rope_q_sin": qs,
            "rope_k_cos": kc, "rope_k_sin": ks,
            "kvvalid": kvvalid,
            "ident8": ident8,
            **wts,
            **masks,
        })
    return ins


_PROG_CACHE = {}


def get_program():
    if "nc" not in _PROG_CACHE:
        _PROG_CACHE["nc"] = build_program()
    return _PROG_CACHE["nc"]


def kernel(x, Wq, Wk, Wv, Wo):
    nc = get_program()
    ins = make_in_maps(x, Wq, Wk, Wv, Wo)
    res = run_bass_kernel_spmd(nc, ins, list(range(NCORE)))
    out = np.empty((B, T, C), np.float32)
    for c in range(NCORE):
        b, ch = divmod(c, 4)
        out[b, CH * ch:CH * (ch + 1), :] = res.results[c]["out"].astype(
            np.float32)
    return out
